# revision 18
# baseline (speedup 1.0000x reference)
"""Bidirectional-ALiBi bias kernel for Trainium2 (Bass/Tile), 8-core SPMD.

Computes out[h, i, j] = |j - i| * m where m = alpha[h] on the first
row/column, gamma[h] above the diagonal, beta[h] below it, and 0 on the
(non-edge) diagonal.  Output [16, 2048, 2048] f32, sharded 2 heads/core.

Strategy: every interior row i is a shifted window of the per-head
profile V(k) = gamma*max(k,0) + beta*max(-k,0), k = j - i.  Each core
computes, per head, THREE overlapping 2047-col diagonalized tiles
W[p, c-lo] = V(c - p - 2047) with lo in {0, 1024, 2048}.  The overlap
is chosen so that for every 128-row block t:
  right half (cols 1024..2047) = ONE contiguous slice of W_B (t>=8)
      or W_C (t<8)  -> DMA'd directly, zero copies;
  left half (cols 0..1023) = one slice of W_A/W_B/W_C plus col 0
      (alpha*i) -> assembled by a single DVE copy + tiny patches into
      a QL tile, then DMA'd.
Block 0 (row 0 = alpha*j) gets both halves assembled with its row-0
patched; all other halves read shared tiles.  Every DMA descriptor in
the kernel is a 4 KB row: HBM write efficiency at 8 KB row stride is
set by descriptor size (4-8 KB descriptors sustain ~400 GB/s; sub-1 KB
descriptors pay a DRAM row-activation per row, <512 B an SDMA
read-modify-write, collapsing throughput to 150-250 GB/s).

Engine placement (respects the DVE/GpSimd shared-SBUF-port lock, and
keeps both HWDGE rings' trigger queues self-paced):
  gpsimd: one master K iota, coefficient partition_broadcasts, IB iota
  ACT (nc.scalar): K chunk derives (K +- 1024), ALL T2 = relu(gamma*K)
      ops (ACT contends with nobody), right-half DMA triggers
  DVE: W = max(-beta*K, T2), left-half assembly, patches
  SP ring (nc.sync): coefficient loads + left-half DMA triggers
Left and right streams are 16.8 MB each - the rings stay balanced.

Hardware notes (from NTFF profiling): 16 SDMA engines; HBM-per-core
limit ~358-420 GB/s; 33.6 MB/core of output writes set a ~84-94 us
roofline, plus ~6.5 us fixed NEFF start barrier, ~7 us of pipeline
fill (coef DMA latency + first chunk), and ~4 us end-barrier drain.
"""

import numpy as np

H = 16
S = 2048
P = 128
N_CORES = 8
H_LOC = H // N_CORES  # 2 heads per core
NT = S // P  # 16 row blocks per head
HW = 1024  # half-row width
CW = 2047  # chunk width

# chunk lo offsets: A=[0,2047), B=[1024,3071), C=[2048,4095)
LO_A, LO_B, LO_C = 0, 1024, 2048

_NC = None


def _build():
    import concourse.bacc as bacc
    import concourse.mybir as mybir
    from concourse.tile import TileContext

    f32 = mybir.dt.float32
    Copy = mybir.ActivationFunctionType.Copy
    Ident = mybir.ActivationFunctionType.Identity
    Relu = mybir.ActivationFunctionType.Relu
    mult, amax = mybir.AluOpType.mult, mybir.AluOpType.max
    nc = bacc.Bacc("TRN2", target_bir_lowering=False, debug=False)

    alpha_d = nc.dram_tensor("alpha", [H_LOC], f32, kind="ExternalInput").ap()
    beta_d = nc.dram_tensor("beta", [H_LOC], f32, kind="ExternalInput").ap()
    gamma_d = nc.dram_tensor("gamma", [H_LOC], f32, kind="ExternalInput").ap()
    out_d = nc.dram_tensor("out", [H_LOC, S, S], f32, kind="ExternalOutput").ap()

    # left half of block t reads c in [2048-128t, 3071-128t); right half
    # c in [3071-128t, 4095-128t).  Serving chunk (single slice each):
    left_lo = lambda t: LO_C if t == 0 else (LO_B if t < 8 else LO_A)
    right_lo = lambda t: LO_C if t < 8 else LO_B

    with TileContext(nc) as tc:
        with (
            tc.tile_pool(name="coef", bufs=1) as cpool,
            tc.tile_pool(name="kpool", bufs=3) as kpool,
            tc.tile_pool(name="tpool", bufs=1) as tpool,
            tc.tile_pool(name="wpool", bufs=3) as wpool,
            tc.tile_pool(name="qlpool", bufs=6) as qlpool,
            tc.tile_pool(name="qrpool", bufs=1) as qrpool,
        ):
            # --- coefficient loads: partition-broadcast DMAs.  These put a
            # descriptor on every SDMA engine, which makes the completion
            # semaphore fire ~10.3us in; a minimal 2-descriptor DMA leaves
            # 14 engines idle and its semaphore was measured to arrive at
            # ~18.5us, stalling the whole compute chain.
            G2 = cpool.tile([P, H_LOC], f32, tag="G2")
            nc.sync.dma_start(out=G2[:], in_=gamma_d.partition_broadcast(P))
            B2 = cpool.tile([P, H_LOC], f32, tag="B2")
            nc.scalar.dma_start(out=B2[:], in_=beta_d.partition_broadcast(P))
            A2 = cpool.tile([P, H_LOC], f32, tag="A2")
            nc.sync.dma_start(out=A2[:], in_=alpha_d.partition_broadcast(P))

            # --- master K iota first on gpsimd (no input deps).  Emitted in
            # two parts (cols [1023,2047) first) so the first T2/W ops can
            # start ~2us earlier than a single [128,2047] iota would allow.
            # K_B[p, x] = (LO_B + x) - p - 2047
            SPL = 1023  # split point; [SPL:CW] is exactly what right t=8 reads
            KB = kpool.tile([P, CW], f32, tag="K")
            nc.gpsimd.iota(
                KB[:, SPL:CW],
                pattern=[[1, CW - SPL]],
                base=LO_B + SPL - (S - 1),
                channel_multiplier=-1,
                allow_small_or_imprecise_dtypes=True,
            )
            nc.gpsimd.iota(
                KB[:, 0:SPL],
                pattern=[[1, SPL]],
                base=LO_B - (S - 1),
                channel_multiplier=-1,
                allow_small_or_imprecise_dtypes=True,
            )
            IB = cpool.tile([P, NT], f32, tag="IB")
            nc.gpsimd.iota(
                IB[:],
                pattern=[[P, NT]],
                base=0,
                channel_multiplier=1,
                allow_small_or_imprecise_dtypes=True,
            )
            bias_p = cpool.tile([P, 1], f32, tag="bias_p")
            nc.gpsimd.memset(bias_p[:], float(HW))
            bias_n = cpool.tile([P, 1], f32, tag="bias_n")
            nc.gpsimd.memset(bias_n[:], float(-HW))

            NB2 = cpool.tile([P, H_LOC], f32, tag="NB2")
            nc.vector.tensor_scalar_mul(NB2[:], B2[:], -1.0)
            Rs = {}
            for h in range(H_LOC):
                Rh = cpool.tile([P, NT], f32, tag=f"Rs{h}")
                nc.vector.tensor_scalar_mul(Rh[:], IB[:], A2[:, h : h + 1])
                Rs[h] = Rh

            Ks = {LO_B: KB}
            Ws = {}

            def derive_k(lo, bias):  # ACT: K_lo = K_B + (lo - LO_B)
                Kg = kpool.tile([P, CW], f32, tag="K")
                nc.scalar.activation(out=Kg[:], in_=KB[:], func=Ident, bias=bias[:])
                Ks[lo] = Kg

            def t2(lo, h, x0=0, x1=CW, T2t=None):  # ACT: T2 = relu(gamma * K)
                if T2t is None:
                    T2t = tpool.tile([P, CW], f32, tag=f"T2{h}")
                nc.scalar.activation(
                    out=T2t[:, x0:x1],
                    in_=Ks[lo][:, x0:x1],
                    func=Relu,
                    scale=G2[:, h : h + 1],
                )
                return T2t

            def wop(lo, h, T2t, x0=0, x1=CW):  # DVE: W = max(-beta*K, T2) == V(k)
                Wt = Ws.get((h, lo))
                if Wt is None:
                    Wt = wpool.tile([P, CW], f32, tag=f"W{h}")
                    Ws[(h, lo)] = Wt
                nc.vector.scalar_tensor_tensor(
                    out=Wt[:, x0:x1],
                    in0=Ks[lo][:, x0:x1],
                    scalar=NB2[:, h : h + 1],
                    in1=T2t[:, x0:x1],
                    op0=mult,
                    op1=amax,
                )

            def emit_right_direct(h, t):
                lo = right_lo(t)
                a = 3071 - 128 * t  # c of j=1024
                nc.scalar.dma_start(
                    out=out_d[h, P * t : P * (t + 1), HW:S],
                    in_=Ws[(h, lo)][:, a - lo : a - lo + HW],
                )

            def emit_right0(h):
                # block 0 right half: row 0 must read alpha*j -> assemble.
                # All ops on ACT so the row-0 overwrite is queue-ordered.
                lo = LO_C
                QR = qrpool.tile([P, HW], f32, tag=f"QR{h}")
                nc.scalar.activation(
                    out=QR[:], in_=Ws[(h, lo)][:, 3071 - lo : 3071 - lo + HW], func=Copy
                )
                nc.scalar.activation(
                    out=QR[0:1, :],
                    in_=Ks[lo][0:1, 3071 - lo : 3071 - lo + HW],
                    func=Copy,
                    scale=A2[0:1, h : h + 1],
                )
                nc.scalar.dma_start(out=out_d[h, 0:P, HW:S], in_=QR[:])

            def emit_left(h, t):
                # col 0 = alpha*i, cols 1..1023 = W slice; t=0 row 0 = alpha*j
                lo = left_lo(t)
                a = 2048 - 128 * t  # c of j=1
                QL = qlpool.tile([P, HW], f32, tag=f"QL{h}")
                nc.vector.tensor_copy(
                    out=QL[:, 1:HW], in_=Ws[(h, lo)][:, a - lo : a - lo + HW - 1]
                )
                if t == 0:
                    nc.vector.tensor_scalar_mul(
                        QL[0:1, 1:HW],
                        Ks[lo][0:1, a - lo : a - lo + HW - 1],
                        A2[0:1, h : h + 1],
                    )
                nc.vector.tensor_copy(out=QL[:, 0:1], in_=Rs[h][:, t : t + 1])
                nc.sync.dma_start(out=out_d[h, P * t : P * (t + 1), 0:HW], in_=QL[:])

            # --- schedule (code order == per-engine queue order) ---
            # h0/B computed in two column halves so right t=8 (which reads
            # exactly W_B[:, SPL:CW]) can launch ~3us earlier.
            T2b0 = t2(LO_B, 0, SPL, CW)
            wop(LO_B, 0, T2b0, SPL, CW)
            t2(LO_B, 0, 0, SPL, T2t=T2b0)
            emit_right_direct(0, 8)
            wop(LO_B, 0, T2b0, 0, SPL)
            for t in range(9, NT):  # remaining rights h0 from W_B (ACT ring)
                emit_right_direct(0, t)
            for t in range(1, 8):  # lefts t=1..7 h0 (from W_B, SP ring) --
                emit_left(0, t)  # pumps the second ring during the ramp
            T2b1 = t2(LO_B, 1)
            wop(LO_B, 1, T2b1)
            for t in range(8, NT):  # rights h1
                emit_right_direct(1, t)
            derive_k(LO_C, bias_p)
            T2c0 = t2(LO_C, 0)
            wop(LO_C, 0, T2c0)
            for t in range(1, 8):  # lefts t=1..7 h1 (from W_B)
                emit_left(1, t)
            T2c1 = t2(LO_C, 1)
            wop(LO_C, 1, T2c1)
            for t in range(1, 8):  # rights t=1..7 h0: direct from W_C
                emit_right_direct(0, t)
            emit_right0(0)
            derive_k(LO_A, bias_n)
            T2a0 = t2(LO_A, 0)
            emit_left(0, 0)
            emit_left(1, 0)
            wop(LO_A, 0, T2a0)
            for t in range(1, 8):  # rights h1
                emit_right_direct(1, t)
            emit_right0(1)
            T2a1 = t2(LO_A, 1)
            wop(LO_A, 1, T2a1)
            for t in range(8, NT):  # lefts t=8..15 (from W_A)
                emit_left(0, t)
            for t in range(8, NT):
                emit_left(1, t)

    nc.compile()
    return nc


def _run(alpha, beta, gamma, **spmd_kwargs):
    """Compile (cached) and run on the 8 NeuronCores; returns BassKernelResults."""
    global _NC
    if _NC is None:
        _NC = _build()
    from concourse import bass_utils

    alpha = np.ascontiguousarray(alpha, dtype=np.float32)
    beta = np.ascontiguousarray(beta, dtype=np.float32)
    gamma = np.ascontiguousarray(gamma, dtype=np.float32)
    in_maps = [
        {
            "alpha": alpha[c * H_LOC : (c + 1) * H_LOC],
            "beta": beta[c * H_LOC : (c + 1) * H_LOC],
            "gamma": gamma[c * H_LOC : (c + 1) * H_LOC],
        }
        for c in range(N_CORES)
    ]
    return bass_utils.run_bass_kernel_spmd(
        _NC, in_maps, core_ids=list(range(N_CORES)), **spmd_kwargs
    )


def kernel(alpha, beta, gamma, seq_len):
    assert int(seq_len) == S, f"kernel hardcodes seq_len={S}, got {seq_len}"
    res = _run(alpha, beta, gamma)
    return np.concatenate([r["out"] for r in res.results], axis=0)


# revision 19
# speedup vs baseline: 1.0130x; 1.0130x over previous
"""Bidirectional-ALiBi bias kernel for Trainium2 (Bass/Tile), 8-core SPMD.

Computes out[h, i, j] = |j - i| * m where m = alpha[h] on the first
row/column, gamma[h] above the diagonal, beta[h] below it, and 0 on the
(non-edge) diagonal.  Output [16, 2048, 2048] f32, sharded 2 heads/core.

Strategy: every interior row i is a shifted window of the per-head
profile V(k) = gamma*max(k,0) + beta*max(-k,0), k = j - i.  Each core
computes, per head, THREE overlapping 2047-col diagonalized tiles
W[p, c-lo] = V(c - p - 2047) with lo in {0, 1024, 2048}.  The overlap
is chosen so that for every 128-row block t:
  right half (cols 1024..2047) = ONE contiguous slice of W_B (t>=8)
      or W_C (t<8)  -> DMA'd directly, zero copies;
  left half (cols 0..1023) = one slice of W_A/W_B/W_C plus col 0
      (alpha*i) -> assembled by a single DVE copy + tiny patches into
      a QL tile, then DMA'd.
Block 0 (row 0 = alpha*j) gets both halves assembled with its row-0
patched; all other halves read shared tiles.  Every DMA descriptor in
the kernel is a 4 KB row: HBM write efficiency at 8 KB row stride is
set by descriptor size (4-8 KB descriptors sustain ~400 GB/s; sub-1 KB
descriptors pay a DRAM row-activation per row, <512 B an SDMA
read-modify-write, collapsing throughput to 150-250 GB/s).

Engine placement (respects the DVE/GpSimd shared-SBUF-port lock, and
keeps both HWDGE rings' trigger queues self-paced):
  gpsimd: one master K iota, coefficient partition_broadcasts, IB iota
  ACT (nc.scalar): K chunk derives (K +- 1024), ALL T2 = relu(gamma*K)
      ops (ACT contends with nobody), right-half DMA triggers
  DVE: W = max(-beta*K, T2), left-half assembly, patches
  SP ring (nc.sync): coefficient loads + left-half DMA triggers
Left and right streams are 16.8 MB each - the rings stay balanced.

Hardware notes (from NTFF profiling): the 16 SDMA engines sustain
419.5 GB/s aggregate and run 100% busy for the whole stream, so the
33.6 MB/core of output writes take 80.0 us flat.  Measured budget:
6.5 us fixed NEFF start barrier + 6.7 us fill (coefficient-broadcast
DMA semaphore ~3.8 us post-barrier, then T2+W on the first 1024 cols)
+ 80.0 us stream + 2.9 us end barrier = ~102.4 us, which is what the
kernel measures (102.3-103.4 us; slower samples track co-tenant HBM
contention).  Baseline before this rewrite: 125.5 us.

Failed approaches worth remembering: T2 on gpsimd 2x'd the runtime
(DVE 2-port perf-mode ops and gpsimd fully block each other on the
shared SBUF port pair); 516 B strip descriptors still dented
throughput ~20% (DRAM row-activation per row); minimal 2-descriptor
coefficient DMAs left 14 SDMA engines out of the semaphore update and
the completion fired ~9 us late.
"""

import numpy as np

H = 16
S = 2048
P = 128
N_CORES = 8
H_LOC = H // N_CORES  # 2 heads per core
NT = S // P  # 16 row blocks per head
HW = 1024  # half-row width
CW = 2047  # chunk width

# chunk lo offsets: A=[0,2047), B=[1024,3071), C=[2048,4095)
LO_A, LO_B, LO_C = 0, 1024, 2048

_NC = None


def _build():
    import concourse.bacc as bacc
    import concourse.mybir as mybir
    from concourse.tile import TileContext

    f32 = mybir.dt.float32
    Copy = mybir.ActivationFunctionType.Copy
    Ident = mybir.ActivationFunctionType.Identity
    Relu = mybir.ActivationFunctionType.Relu
    mult, amax = mybir.AluOpType.mult, mybir.AluOpType.max
    nc = bacc.Bacc("TRN2", target_bir_lowering=False, debug=False)

    alpha_d = nc.dram_tensor("alpha", [H_LOC], f32, kind="ExternalInput").ap()
    beta_d = nc.dram_tensor("beta", [H_LOC], f32, kind="ExternalInput").ap()
    gamma_d = nc.dram_tensor("gamma", [H_LOC], f32, kind="ExternalInput").ap()
    out_d = nc.dram_tensor("out", [H_LOC, S, S], f32, kind="ExternalOutput").ap()

    # left half of block t reads c in [2048-128t, 3071-128t); right half
    # c in [3071-128t, 4095-128t).  Serving chunk (single slice each):
    left_lo = lambda t: LO_C if t == 0 else (LO_B if t < 8 else LO_A)
    right_lo = lambda t: LO_C if t < 8 else LO_B

    with TileContext(nc) as tc:
        with (
            tc.tile_pool(name="coef", bufs=1) as cpool,
            tc.tile_pool(name="kpool", bufs=3) as kpool,
            tc.tile_pool(name="tpool", bufs=1) as tpool,
            tc.tile_pool(name="wpool", bufs=3) as wpool,
            tc.tile_pool(name="qlpool", bufs=6) as qlpool,
            tc.tile_pool(name="qrpool", bufs=1) as qrpool,
        ):
            # --- coefficient loads: partition-broadcast DMAs.  These put a
            # descriptor on every SDMA engine, which makes the completion
            # semaphore fire ~10.3us in; a minimal 2-descriptor DMA leaves
            # 14 engines idle and its semaphore was measured to arrive at
            # ~18.5us, stalling the whole compute chain.
            G2 = cpool.tile([P, H_LOC], f32, tag="G2")
            nc.sync.dma_start(out=G2[:], in_=gamma_d.partition_broadcast(P))
            B2 = cpool.tile([P, H_LOC], f32, tag="B2")
            nc.scalar.dma_start(out=B2[:], in_=beta_d.partition_broadcast(P))
            A2 = cpool.tile([P, H_LOC], f32, tag="A2")
            nc.sync.dma_start(out=A2[:], in_=alpha_d.partition_broadcast(P))

            # --- master K iota first on gpsimd (no input deps).  Emitted in
            # two parts (cols [1023,2047) first) so the first T2/W ops can
            # start ~2us earlier than a single [128,2047] iota would allow.
            # K_B[p, x] = (LO_B + x) - p - 2047
            SPL = 1023  # split point; [SPL:CW] is exactly what right t=8 reads
            KB = kpool.tile([P, CW], f32, tag="K")
            nc.gpsimd.iota(
                KB[:, SPL:CW],
                pattern=[[1, CW - SPL]],
                base=LO_B + SPL - (S - 1),
                channel_multiplier=-1,
                allow_small_or_imprecise_dtypes=True,
            )
            nc.gpsimd.iota(
                KB[:, 0:SPL],
                pattern=[[1, SPL]],
                base=LO_B - (S - 1),
                channel_multiplier=-1,
                allow_small_or_imprecise_dtypes=True,
            )
            IB = cpool.tile([P, NT], f32, tag="IB")
            nc.gpsimd.iota(
                IB[:],
                pattern=[[P, NT]],
                base=0,
                channel_multiplier=1,
                allow_small_or_imprecise_dtypes=True,
            )
            bias_p = cpool.tile([P, 1], f32, tag="bias_p")
            nc.gpsimd.memset(bias_p[:], float(HW))
            bias_n = cpool.tile([P, 1], f32, tag="bias_n")
            nc.gpsimd.memset(bias_n[:], float(-HW))

            NB2 = cpool.tile([P, H_LOC], f32, tag="NB2")
            nc.vector.tensor_scalar_mul(NB2[:], B2[:], -1.0)
            Rs = {}
            for h in range(H_LOC):
                Rh = cpool.tile([P, NT], f32, tag=f"Rs{h}")
                nc.vector.tensor_scalar_mul(Rh[:], IB[:], A2[:, h : h + 1])
                Rs[h] = Rh

            Ks = {LO_B: KB}
            Ws = {}

            def derive_k(lo, bias):  # ACT: K_lo = K_B + (lo - LO_B)
                Kg = kpool.tile([P, CW], f32, tag="K")
                nc.scalar.activation(out=Kg[:], in_=KB[:], func=Ident, bias=bias[:])
                Ks[lo] = Kg

            def t2(lo, h, x0=0, x1=CW, T2t=None):  # ACT: T2 = relu(gamma * K)
                if T2t is None:
                    T2t = tpool.tile([P, CW], f32, tag=f"T2{h}")
                nc.scalar.activation(
                    out=T2t[:, x0:x1],
                    in_=Ks[lo][:, x0:x1],
                    func=Relu,
                    scale=G2[:, h : h + 1],
                )
                return T2t

            def wop(lo, h, T2t, x0=0, x1=CW):  # DVE: W = max(-beta*K, T2) == V(k)
                Wt = Ws.get((h, lo))
                if Wt is None:
                    Wt = wpool.tile([P, CW], f32, tag=f"W{h}")
                    Ws[(h, lo)] = Wt
                nc.vector.scalar_tensor_tensor(
                    out=Wt[:, x0:x1],
                    in0=Ks[lo][:, x0:x1],
                    scalar=NB2[:, h : h + 1],
                    in1=T2t[:, x0:x1],
                    op0=mult,
                    op1=amax,
                )

            def emit_right_direct(h, t):
                lo = right_lo(t)
                a = 3071 - 128 * t  # c of j=1024
                nc.scalar.dma_start(
                    out=out_d[h, P * t : P * (t + 1), HW:S],
                    in_=Ws[(h, lo)][:, a - lo : a - lo + HW],
                )

            def emit_right0(h):
                # block 0 right half: row 0 must read alpha*j -> assemble.
                # All ops on ACT so the row-0 overwrite is queue-ordered.
                lo = LO_C
                QR = qrpool.tile([P, HW], f32, tag=f"QR{h}")
                nc.scalar.activation(
                    out=QR[:], in_=Ws[(h, lo)][:, 3071 - lo : 3071 - lo + HW], func=Copy
                )
                nc.scalar.activation(
                    out=QR[0:1, :],
                    in_=Ks[lo][0:1, 3071 - lo : 3071 - lo + HW],
                    func=Copy,
                    scale=A2[0:1, h : h + 1],
                )
                nc.scalar.dma_start(out=out_d[h, 0:P, HW:S], in_=QR[:])

            def emit_left(h, t):
                # col 0 = alpha*i, cols 1..1023 = W slice; t=0 row 0 = alpha*j
                lo = left_lo(t)
                a = 2048 - 128 * t  # c of j=1
                QL = qlpool.tile([P, HW], f32, tag=f"QL{h}")
                nc.vector.tensor_copy(
                    out=QL[:, 1:HW], in_=Ws[(h, lo)][:, a - lo : a - lo + HW - 1]
                )
                if t == 0:
                    nc.vector.tensor_scalar_mul(
                        QL[0:1, 1:HW],
                        Ks[lo][0:1, a - lo : a - lo + HW - 1],
                        A2[0:1, h : h + 1],
                    )
                nc.vector.tensor_copy(out=QL[:, 0:1], in_=Rs[h][:, t : t + 1])
                nc.sync.dma_start(out=out_d[h, P * t : P * (t + 1), 0:HW], in_=QL[:])

            # --- schedule (code order == per-engine queue order) ---
            # h0/B computed in two column halves so right t=8 (which reads
            # exactly W_B[:, SPL:CW]) can launch ~3us earlier.
            T2b0 = t2(LO_B, 0, SPL, CW)
            wop(LO_B, 0, T2b0, SPL, CW)
            t2(LO_B, 0, 0, SPL, T2t=T2b0)
            emit_right_direct(0, 8)
            wop(LO_B, 0, T2b0, 0, SPL)
            for t in range(9, NT):  # remaining rights h0 from W_B (ACT ring)
                emit_right_direct(0, t)
            for t in range(1, 8):  # lefts t=1..7 h0 (from W_B, SP ring) --
                emit_left(0, t)  # pumps the second ring during the ramp
            T2b1 = t2(LO_B, 1)
            wop(LO_B, 1, T2b1)
            for t in range(8, NT):  # rights h1
                emit_right_direct(1, t)
            derive_k(LO_C, bias_p)
            T2c0 = t2(LO_C, 0)
            wop(LO_C, 0, T2c0)
            for t in range(1, 8):  # lefts t=1..7 h1 (from W_B)
                emit_left(1, t)
            T2c1 = t2(LO_C, 1)
            wop(LO_C, 1, T2c1)
            for t in range(1, 8):  # rights t=1..7 h0: direct from W_C
                emit_right_direct(0, t)
            emit_right0(0)
            derive_k(LO_A, bias_n)
            T2a0 = t2(LO_A, 0)
            emit_left(0, 0)
            emit_left(1, 0)
            wop(LO_A, 0, T2a0)
            for t in range(1, 8):  # rights h1
                emit_right_direct(1, t)
            emit_right0(1)
            T2a1 = t2(LO_A, 1)
            wop(LO_A, 1, T2a1)
            for t in range(8, NT):  # lefts t=8..15 (from W_A)
                emit_left(0, t)
            for t in range(8, NT):
                emit_left(1, t)

    nc.compile()
    return nc


def _run(alpha, beta, gamma, **spmd_kwargs):
    """Compile (cached) and run on the 8 NeuronCores; returns BassKernelResults."""
    global _NC
    if _NC is None:
        _NC = _build()
    from concourse import bass_utils

    alpha = np.ascontiguousarray(alpha, dtype=np.float32)
    beta = np.ascontiguousarray(beta, dtype=np.float32)
    gamma = np.ascontiguousarray(gamma, dtype=np.float32)
    in_maps = [
        {
            "alpha": alpha[c * H_LOC : (c + 1) * H_LOC],
            "beta": beta[c * H_LOC : (c + 1) * H_LOC],
            "gamma": gamma[c * H_LOC : (c + 1) * H_LOC],
        }
        for c in range(N_CORES)
    ]
    return bass_utils.run_bass_kernel_spmd(
        _NC, in_maps, core_ids=list(range(N_CORES)), **spmd_kwargs
    )


def kernel(alpha, beta, gamma, seq_len):
    assert int(seq_len) == S, f"kernel hardcodes seq_len={S}, got {seq_len}"
    res = _run(alpha, beta, gamma)
    return np.concatenate([r["out"] for r in res.results], axis=0)
